# revision 22
# baseline (speedup 1.0000x reference)
"""MoE transformer block on 8 TRN2 NeuronCores.

Sharding: data-parallel over batch (4 batches = 784 tokens per core), no
collectives.  On-chip layout is feature-major ([d, tokens]) everywhere; the
host pre-transposes q/k/v to [D, T] and un-transposes the [D, T] output, so
the device does no layout transposes on the IO path.

MoE runs in fp8e4 DoubleRow (2 fp8 MACs/cell): the host pre-casts W1/W2 to
fp8 and pre-interleaves them into the [128, kpair, free] stationary layout,
so weight DMA is 37.7 MB/core of contiguous 18 KB/partition reads.  x and h
are cast to fp8 on-chip (DVE / ACT evictions).  Attention stays fp32r.

PSUM discipline: two pools, one unified tag each (every psum tile <= 1 bank,
4 bufs per pool -> exactly 8 banks).  The MoE y-phase holds 2+2 accumulators
across the K(=F) loop while the h-phase double-buffers 1+1.
"""
import sys

sys.path.insert(0, "/opt/trn_rl_repo")

from contextlib import ExitStack

import ml_dtypes
import numpy as np

import concourse.bass as bass
import concourse.tile as tile
from concourse import bacc, mybir
from concourse.bass_utils import run_bass_kernel_spmd

FP32 = mybir.dt.float32
FP32R = mybir.dt.float32r
FP8 = mybir.dt.float8e4
DR = mybir.MatmulPerfMode.DoubleRow
AF = mybir.ActivationFunctionType
OP = mybir.AluOpType

B, S, D, H, E, F = 32, 196, 768, 12, 8, 3072
DH = D // H                 # 64
NCORES = 8
BPC = B // NCORES           # 4 batches per core
T = BPC * S                 # 784 tokens per core
TH = T // 2                 # 392 tokens per half (2 batches)
DK = D // 128               # 6
FK = F // 128               # 24
EPS = 1e-5
BK = [(0, 128), (128, 68)]                              # ki chunks per batch
NCH = ((0, TH), (TH, TH))                               # token halves
MCH = ((0, 512), (512, T - 512))                        # moe chunks (16B align)
# packed bias/scale column indices in colpack
CBQ, CBK, CBO, CBV, CL1G, CL1B, CL2G, CL2B = range(8)

_CACHE = {}


def _build(skip_attn=False, skip_moe=False, repeat=1):
    nc = bacc.Bacc("TRN2", target_bir_lowering=False, debug=False,
                   num_devices=NCORES)

    qT_d = nc.dram_tensor("qT", [D, T], FP32R, kind="ExternalInput").ap()
    kT_d = nc.dram_tensor("kT", [D, T], FP32R, kind="ExternalInput").ap()
    vT_d = nc.dram_tensor("vT", [D, T], FP32R, kind="ExternalInput").ap()
    wq_d = nc.dram_tensor("Wq", [D, D], FP32R, kind="ExternalInput").ap()
    wk_d = nc.dram_tensor("Wk", [D, D], FP32R, kind="ExternalInput").ap()
    wv_d = nc.dram_tensor("Wv", [D, D], FP32R, kind="ExternalInput").ap()
    wo_d = nc.dram_tensor("Wo", [D, D], FP32R, kind="ExternalInput").ap()
    cols_d = nc.dram_tensor("colpack", [128, 8, DK], FP32,
                            kind="ExternalInput").ap()
    wgp_d = nc.dram_tensor("wgp", [128, DK, E], FP32R,
                           kind="ExternalInput").ap()
    bg_d = nc.dram_tensor("bg", [E], FP32, kind="ExternalInput").ap()
    b1t_d = nc.dram_tensor("b1T", [E, 128, FK], FP32,
                           kind="ExternalInput").ap()
    b2r_d = nc.dram_tensor("b2r", [E, D], FP32R, kind="ExternalInput").ap()
    w1f8_d = nc.dram_tensor("w1f8", [E, 128, DK, F], FP8,
                            kind="ExternalInput").ap()
    w2f8_d = nc.dram_tensor("w2f8", [E, 128, FK, D], FP8,
                            kind="ExternalInput").ap()
    sel_d = nc.dram_tensor("sel8", [E, E * 128], FP32R,
                           kind="ExternalInput").ap()
    aux1_d = nc.dram_tensor("aux_ones", [128, 128], FP32,
                            kind="ExternalInput").ap()
    aux1r_d = nc.dram_tensor("aux_onesr", [128, 128], FP32R,
                             kind="ExternalInput").ap()
    zo_d = nc.dram_tensor("zo2", [2, 128], FP32R, kind="ExternalInput").ap()
    auxe_d = nc.dram_tensor("aux_eps", [1, 1], FP32,
                            kind="ExternalInput").ap()
    out_d = nc.dram_tensor("outT", [D, T], FP32, kind="ExternalOutput").ap()

    with tile.TileContext(nc) as tc, ExitStack() as top:
        const = top.enter_context(tc.tile_pool(name="const", bufs=1))
        vecs = top.enter_context(tc.tile_pool(name="vecs", bufs=1))
        rows = top.enter_context(tc.tile_pool(name="rows", bufs=2))
        psA = top.enter_context(tc.tile_pool(name="psA", bufs=4, space="PSUM"))
        psB = top.enter_context(tc.tile_pool(name="psB", bufs=4, space="PSUM"))
        tmp = top.enter_context(tc.tile_pool(name="tmp", bufs=2))
        persist = top.enter_context(tc.tile_pool(name="persist", bufs=1))

        def pa(p, f):
            return psA.tile([p, f], FP32, tag="a", name="pa")

        def pb(p, f):
            return psB.tile([p, f], FP32, tag="b", name="pb")

        # ---------------- constants ----------------
        ones_col_r = const.tile([128, 1], FP32R, tag="ones_col_r")
        nc.sync.dma_start(out=ones_col_r[:], in_=aux1r_d[:, 0:1])
        ones_row_r = const.tile([1, 128], FP32R, tag="ones_row_r")
        nc.sync.dma_start(out=ones_row_r[:], in_=aux1r_d[0:1, :])
        ones_row8_r = const.tile([1, 8], FP32R, tag="ones_row8_r")
        nc.sync.dma_start(out=ones_row8_r[:], in_=aux1r_d[0:1, 0:8])
        ones8_col = const.tile([8, 1], FP32, tag="ones8_col")
        nc.sync.dma_start(out=ones8_col[:], in_=aux1_d[0:8, 0:1])
        # mask rows for the 1/den broadcast: row0 -> cols 0:64, row32 -> 64:128
        zo33 = const.tile([33, 128], FP32R, tag="zo33")
        nc.sync.dma_start(out=zo33[0:1, :], in_=zo_d[0:1, :])
        nc.sync.dma_start(out=zo33[32:33, :], in_=zo_d[1:2, :])
        eps_t = const.tile([1, 1], FP32, tag="eps")
        nc.sync.dma_start(out=eps_t[:], in_=auxe_d[:, :])
        # per-expert selector: sel8[i, e*128 + p] = (i == e), host-built
        sel8 = const.tile([8, E * 128], FP32R, tag="sel8")
        nc.sync.dma_start(out=sel8[:], in_=sel_d[:, :])

        # packed bias/scale columns: cp[p, i, k] = vec_i[k*128 + p]
        cp = vecs.tile([128, 8, DK], FP32, tag="colpack")
        nc.sync.dma_start(out=cp[:], in_=cols_d[:, :, :])

        def col(i, k):
            return cp[:, i, k:k + 1]

        bg_col = vecs.tile([8, 1], FP32, tag="bg")
        nc.sync.dma_start(out=bg_col[:],
                          in_=bg_d.rearrange("(a b) -> a b", b=1))
        wgs = vecs.tile([128, DK, E], FP32R, tag="wg")
        nc.sync.dma_start(out=wgs[:], in_=wgp_d[:, :, :])
        b2s = vecs.tile([E, D], FP32R, tag="b2")
        nc.sync.dma_start(out=b2s[:], in_=b2r_d[:, :])

        # persistent activations (full T)
        x_t = [persist.tile([128, T], FP32R, tag=f"xt{k}", name=f"xt{k}") for k in range(DK)]
        moe = [persist.tile([128, T], FP32, tag=f"moe{k}", name=f"moe{k}") for k in range(DK)]
        bias_total = vecs.tile([128, DK], FP32, tag="btot")

        def layer_norm(r_tiles, gi, bi, out_tiles, out_off, nch_list):
            # feature-major LN over D=768 partitions (6 tiles); r_tiles fp32r
            for (n0, nl) in nch_list:
                ps_s = pa(1, TH)
                ps_s2 = pa(1, TH)
                sqs = []
                for k in range(DK):
                    sq = tmp.tile([128, TH], FP32R, tag="ln_sq", bufs=6)
                    nc.scalar.activation(sq[:], r_tiles[k][:, n0:n0 + nl],
                                         AF.Square)
                    sqs.append(sq)
                for k in range(DK):
                    nc.tensor.matmul(ps_s[:], ones_col_r[:],
                                     r_tiles[k][:, n0:n0 + nl],
                                     start=(k == 0), stop=(k == DK - 1))
                for k in range(DK):
                    nc.tensor.matmul(ps_s2[:], ones_col_r[:], sqs[k][:],
                                     start=(k == 0), stop=(k == DK - 1))
                m = rows.tile([1, TH], FP32, tag="ln_m", bufs=1)
                m2 = rows.tile([1, TH], FP32, tag="ln_m2", bufs=1)
                nc.vector.tensor_scalar_mul(m[:], ps_s[:], 1.0 / D)
                nc.vector.tensor_scalar_mul(m2[:], ps_s2[:], 1.0 / D)
                mm_ = rows.tile([1, TH], FP32, tag="ln_mm", bufs=1)
                nc.vector.tensor_mul(mm_[:], m[:], m[:])
                var = rows.tile([1, TH], FP32, tag="ln_var", bufs=1)
                nc.vector.tensor_sub(var[:], m2[:], mm_[:])
                sd = rows.tile([1, TH], FP32, tag="ln_sd", bufs=1)
                nc.scalar.activation(sd[:], var[:], AF.Sqrt, bias=eps_t[:])
                rstd = rows.tile([1, TH], FP32R, tag="ln_rstd", bufs=1)
                with nc.allow_low_precision(reason="fp32r matmul operand"):
                    nc.vector.reciprocal(rstd[:], sd[:])
                mr = rows.tile([1, TH], FP32R, tag="ln_mr", bufs=1)
                nc.vector.tensor_mul(mr[:], m[:], rstd[:])
                pR = pb(128, TH)
                nc.tensor.matmul(pR[:], ones_row_r[:], rstd[:],
                                 start=True, stop=True)
                pM = pb(128, TH)
                nc.tensor.matmul(pM[:], ones_row_r[:], mr[:],
                                 start=True, stop=True)
                for k in range(DK):
                    t1 = tmp.tile([128, TH], FP32, tag="ln_t1")
                    nc.vector.tensor_mul(t1[:], r_tiles[k][:, n0:n0 + nl],
                                         pR[:])
                    t2 = tmp.tile([128, TH], FP32, tag="ln_t2")
                    nc.vector.tensor_sub(t2[:], t1[:], pM[:])
                    o0 = out_off + n0
                    nc.scalar.activation(out_tiles[k][:, o0:o0 + nl],
                                         t2[:], AF.Identity,
                                         bias=col(bi, k),
                                         scale=col(gi, k))

        for rep_i in range(repeat):
            # ================= attention, per token-half =================
            if skip_attn:
                for k in range(DK):
                    nc.sync.dma_start(out=x_t[k][:],
                                      in_=qT_d[k * 128:(k + 1) * 128, :])
            for half in range(2 if not skip_attn else 0):
                h0tok = half * TH
                with ExitStack() as hs:
                    ph = hs.enter_context(tc.tile_pool(name=f"ph{half}_{rep_i}", bufs=1))
                    phw = hs.enter_context(tc.tile_pool(name=f"phw{half}_{rep_i}", bufs=6))
                    phe = hs.enter_context(tc.tile_pool(name=f"phe{half}_{rep_i}", bufs=4))

                    q_t = [ph.tile([128, TH], FP32R, tag=f"qt{k}", name=f"qt{k}")
                           for k in range(DK)]
                    k_t = [ph.tile([128, TH], FP32R, tag=f"kt{k}", name=f"kt{k}")
                           for k in range(DK)]
                    v_t = [ph.tile([128, TH], FP32R, tag=f"vt{k}", name=f"vt{k}")
                           for k in range(DK)]
                    # ---- loads, in dependency-priority order: the sync DMA
                    # ring is FIFO, so q+Wq go first (first matmuls need only
                    # them), then k+Wk, then v+Wv.
                    wq = []
                    wk = []
                    wv = []
                    for dram, dst, wdram, wlist in (
                            (qT_d, q_t, wq_d, wq), (kT_d, k_t, wk_d, wk),
                            (vT_d, v_t, wv_d, wv)):
                        for k in range(DK):
                            nc.sync.dma_start(
                                out=dst[k][:],
                                in_=dram[k * 128:(k + 1) * 128,
                                         h0tok:h0tok + TH])
                        for k in range(DK):
                            wt = phw.tile([128, D], FP32R, tag="wproj", name="wt")
                            nc.sync.dma_start(
                                out=wt[:], in_=wdram[k * 128:(k + 1) * 128, :])
                            wlist.append(wt)

                    # ---- qh/kh projections (fp32 out, feed fp32 scores) ----
                    qh_t = [ph.tile([128, TH], FP32R, tag=f"qh{k}", name=f"qh{k}")
                            for k in range(DK)]
                    kh_t = [ph.tile([128, TH], FP32R, tag=f"kh{k}", name=f"kh{k}")
                            for k in range(DK)]
                    for w, src, dst, bci in ((wq, q_t, qh_t, CBQ),
                                             (wk, k_t, kh_t, CBK)):
                        for mi in range(DK):
                            ps = pa(128, TH)
                            for k in range(DK):
                                nc.tensor.matmul(
                                    ps[:], w[k][:, mi * 128:(mi + 1) * 128],
                                    src[k][:], start=(k == 0), stop=(k == DK - 1))
                            nc.scalar.activation(dst[mi][:], ps[:], AF.Identity,
                                                 bias=col(bci, mi))

                    # ---- vh token-major per (batch, ki-chunk), zero-padded:
                    # head hh occupies cols (hh%2)*64..+64 of its 128-wide
                    # slice, the other half stays zero so even/odd heads can
                    # share a ctx PSUM bank via accumulation ----
                    vh = {}
                    for bl in range(2):
                        for ci, (c0, cl) in enumerate(BK):
                            vt_ = ph.tile([128, H, 128], FP32R,
                                          tag=f"vh{bl}{ci}", name=f"vh{bl}{ci}")
                            nc.vector.memset(
                                vt_[:].rearrange("p h f -> p (h f)")
                                .bitcast(FP32), 0.0)
                            tc0 = bl * S + c0
                            for ni in range(2):
                                ps = pa(128, 384)
                                for k in range(DK):
                                    nc.tensor.matmul(
                                        ps[:cl, :], v_t[k][:, tc0:tc0 + cl],
                                        wv[k][:, ni * 384:(ni + 1) * 384],
                                        start=(k == 0), stop=(k == DK - 1))
                                src = ps[:cl, :].rearrange(
                                    "p (j two d) -> p j two d", two=2, d=DH)
                                dst = vt_[:cl, ni * 6:(ni + 1) * 6, :] \
                                    .rearrange("p (j two) f -> p j two f",
                                               two=2)
                                for par in range(2):
                                    nc.vector.tensor_copy(
                                        dst[:, :, par, par * DH:(par + 1) * DH],
                                        src[:, :, par, :])
                            vh[(bl, ci)] = vt_

                    # ---- attention, head pairs sharing a ctx PSUM bank ----
                    # even head writes all 128 rows (values + zero half),
                    # odd head accumulates its half on top
                    cxp = [ph.tile([128, TH], FP32R, tag=f"cx{mi}", name=f"cx{mi}")
                           for mi in range(DK)]
                    for dm in range(DK):
                        ctxs = (pa(128, TH), pa(128, TH))
                        pdens = []
                        for sub in range(2):
                            hh = dm * 2 + sub
                            ro = sub * DH
                            pden = pb(1, TH)
                            for bl in range(2):
                                bc = bl * S
                                exps = []
                                for ci, (c0, cl) in enumerate(BK):
                                    # scores^T: batch bl keys vs BOTH batches'
                                    # queries (cross half garbage, never read)
                                    ps = pa(128, TH)
                                    nc.tensor.matmul(
                                        ps[:cl, :],
                                        kh_t[dm][ro:ro + DH,
                                                 bc + c0:bc + c0 + cl],
                                        qh_t[dm][ro:ro + DH, :],
                                        start=True, stop=True)
                                    ex = phe.tile([128, TH], FP32R, tag="exp",
                                                  bufs=3)
                                    nc.scalar.activation(ex[:cl, :], ps[:cl, :],
                                                         AF.Exp, scale=0.125)
                                    exps.append((ex, cl))
                                for ci, (ex, cl) in enumerate(exps):
                                    nc.tensor.matmul(
                                        ctxs[bl][:],
                                        vh[(bl, ci)][:cl, hh, :], ex[:cl, :],
                                        start=(sub == 0 and ci == 0),
                                        stop=(sub == 1 and ci == 1),
                                        skip_group_check=True)
                                for ci, (ex, cl) in enumerate(exps):
                                    nc.tensor.matmul(
                                        pden[0:1, bc:bc + S], ones_col_r[:cl],
                                        ex[:cl, bc:bc + S],
                                        start=(ci == 0), stop=(ci == 1),
                                        skip_group_check=True)
                            pdens.append(pden)
                        # both heads' denominators -> rows 0 / 32, broadcast
                        # via mask rows, one full-width reciprocal
                        dno = rows.tile([33, TH], FP32R, tag="dno", bufs=2)
                        nc.scalar.copy(dno[0:1, :], pdens[0][:])
                        nc.scalar.copy(dno[32:33, :], pdens[1][:])
                        pbc = pb(128, TH)
                        nc.tensor.matmul(pbc[:], zo33[0:1, :], dno[0:1, :],
                                         start=True, stop=False)
                        nc.tensor.matmul(pbc[:], zo33[32:33, :], dno[32:33, :],
                                         start=False, stop=True)
                        rdenb = phe.tile([128, TH], FP32, tag="rdenb", bufs=2)
                        nc.vector.reciprocal(rdenb[:], pbc[:])
                        for bl in range(2):
                            bc = bl * S
                            nc.vector.tensor_mul(cxp[dm][:, bc:bc + S],
                                                 ctxs[bl][:, bc:bc + S],
                                                 rdenb[:, bc:bc + S])

                    # ---- Wo projection + bias_total + residual -> r1 ----
                    wo = []
                    for k in range(DK):
                        wt = phw.tile([128, D], FP32R, tag="wproj", name="wt")
                        nc.sync.dma_start(
                            out=wt[:], in_=wo_d[k * 128:(k + 1) * 128, :])
                        wo.append(wt)
                    if half == 0:
                        for mi in range(DK):
                            pbs = pb(128, 1)
                            for k in range(DK):
                                nc.tensor.matmul(
                                    pbs[:],
                                    wo[k][:, mi * 128:(mi + 1) * 128].bitcast(
                                        FP32),
                                    col(CBV, k),
                                    start=(k == 0), stop=(k == DK - 1))
                            nc.vector.tensor_add(bias_total[:, mi:mi + 1], pbs[:],
                                                 col(CBO, mi))
                    r1 = [ph.tile([128, TH], FP32R, tag=f"r1{mi}", name=f"r1{mi}")
                          for mi in range(DK)]
                    for mi in range(DK):
                        ps = pa(128, TH)
                        for k in range(DK):
                            nc.tensor.matmul(
                                ps[:], wo[k][:, mi * 128:(mi + 1) * 128],
                                cxp[k][:], start=(k == 0), stop=(k == DK - 1))
                        nc.vector.scalar_tensor_tensor(
                            out=r1[mi][:], in0=ps[:],
                            scalar=bias_total[:, mi:mi + 1], in1=q_t[mi][:],
                            op0=OP.add, op1=OP.add)

                    layer_norm(r1, CL1G, CL1B, x_t, h0tok, [(0, TH)])

            # ================= gates =================
            gexp = persist.tile([8, T], FP32, tag="gexp")
            gate = persist.tile([8, T], FP32R, tag="gate")
            for (n0, nl) in NCH:
                pg = pb(8, TH)
                for k in range(DK):
                    nc.tensor.matmul(pg[:], wgs[:, k, :], x_t[k][:, n0:n0 + nl],
                                     start=(k == 0), stop=(k == DK - 1))
                nc.scalar.activation(gexp[:, n0:n0 + nl], pg[:], AF.Exp,
                                     bias=bg_col[:])
                pgs = pb(1, TH)
                nc.tensor.matmul(pgs[:], ones8_col[:], gexp[:, n0:n0 + nl],
                                 start=True, stop=True)
                grec = rows.tile([1, TH], FP32R, tag="grec", bufs=1)
                with nc.allow_low_precision(reason="fp32r matmul operand"):
                    nc.vector.reciprocal(grec[:], pgs[:])
                pgr = pb(8, TH)
                nc.tensor.matmul(pgr[:], ones_row8_r[:], grec[:],
                                 start=True, stop=True)
                nc.vector.tensor_mul(gate[:, n0:n0 + nl], gexp[:, n0:n0 + nl],
                                     pgr[:])

            # moe_acc init = gates^T @ b2   (lhsT = b2 chunks [8, 128])
            for mi in range(DK):
                for (n0, nl) in NCH:
                    pbi = pa(128, TH)
                    nc.tensor.matmul(pbi[:], b2s[:, mi * 128:(mi + 1) * 128],
                                     gate[:, n0:n0 + nl], start=True, stop=True)
                    nc.scalar.copy(moe[mi][:, n0:n0 + nl], pbi[:])

            # ================= MoE experts (fp8e4 DoubleRow) =================
            with ExitStack() as ms:
              if not skip_moe:
                  pmh = ms.enter_context(tc.tile_pool(name=f"pmh_{rep_i}", bufs=2))
                  pmw = ms.enter_context(tc.tile_pool(name=f"pmw_{rep_i}", bufs=2))
                  # x8[p, j, t] = x[j*128+p, t] pair-interleaved for DR rhs
                  x8 = pmh.tile([128, DK, T], FP8, tag="x8", bufs=1, name="x8")
                  with nc.allow_low_precision(reason="fp8 moe operand"):
                      for k in range(DK):
                          nc.vector.tensor_copy(x8[:, k, :],
                                                x_t[k][:].bitcast(FP32))
                  for e in range(E):
                      # weights pre-cast to fp8 + pre-interleaved on the host
                      w1t = pmw.tile([128, DK, F], FP8, tag="w1", bufs=2,
                                     name="w1t")
                      nc.scalar.dma_start(out=w1t[:], in_=w1f8_d[e])
                      w2t = pmw.tile([128, FK, D], FP8, tag="w2", bufs=2,
                                     name="w2t")
                      nc.scalar.dma_start(out=w2t[:], in_=w2f8_d[e])
                      b1c = rows.tile([128, FK], FP32, tag="b1col")
                      nc.sync.dma_start(out=b1c[:], in_=b1t_d[e])

                      # gate row broadcast to 128 partitions, evicted to SBUF
                      grep = tmp.tile([128, T], FP32, tag="gerep")
                      for (n0, nl) in NCH:
                          pge = pb(128, TH)
                          nc.tensor.matmul(pge[:],
                                           sel8[:, e * 128:(e + 1) * 128],
                                           gate[:, n0:n0 + nl],
                                           start=True, stop=True)
                          nc.vector.tensor_copy(grep[:, n0:n0 + nl], pge[:])

                      # ---- h = gelu(W1[e]^T @ x + b1), fp8 [128, FK, T] ----
                      h8 = pmh.tile([128, FK, T], FP8, tag="h8", bufs=1,
                                    name="h8")
                      for fm in range(FK):
                          ph0 = pa(128, MCH[0][1])
                          ph1 = pb(128, MCH[1][1])
                          phs = (ph0, ph1)
                          for j in range(DK // 2):
                              for ni, (n0, nl) in enumerate(MCH):
                                  nc.tensor.matmul(
                                      phs[ni][:],
                                      w1t[:, 2 * j:2 * j + 2,
                                          fm * 128:(fm + 1) * 128],
                                      x8[:, 2 * j:2 * j + 2, n0:n0 + nl],
                                      start=(j == 0), stop=(j == DK // 2 - 1),
                                      perf_mode=DR)
                          for ni, (n0, nl) in enumerate(MCH):
                              nc.scalar.activation(h8[:, fm, n0:n0 + nl],
                                                   phs[ni][:], AF.Gelu,
                                                   bias=b1c[:, fm:fm + 1])

                      # ---- y = W2[e]^T @ h (K-accum in PSUM), combine ----
                      for dg in range(3):
                          pys = [pa(128, MCH[0][1]), pa(128, MCH[1][1]),
                                 pb(128, MCH[0][1]), pb(128, MCH[1][1])]
                          for f2 in range(FK // 2):
                              for j in range(2):
                                  mi = dg * 2 + j
                                  for ni, (n0, nl) in enumerate(MCH):
                                      nc.tensor.matmul(
                                          pys[j * 2 + ni][:],
                                          w2t[:, 2 * f2:2 * f2 + 2,
                                              mi * 128:(mi + 1) * 128],
                                          h8[:, 2 * f2:2 * f2 + 2, n0:n0 + nl],
                                          start=(f2 == 0),
                                          stop=(f2 == FK // 2 - 1),
                                          perf_mode=DR)
                          for j in range(2):
                              mi = dg * 2 + j
                              for ni, (n0, nl) in enumerate(MCH):
                                  ty = tmp.tile([128, nl], FP32, tag="ty")
                                  nc.vector.tensor_mul(ty[:], pys[j * 2 + ni][:],
                                                       grep[:, n0:n0 + nl])
                                  nc.vector.tensor_add(moe[mi][:, n0:n0 + nl],
                                                       moe[mi][:, n0:n0 + nl],
                                                       ty[:])

            # ================= LN2 + output =================
            # r2 = x + moe, written in place into x_t; LN2 output reuses moe
            for mi in range(DK):
                nc.vector.tensor_add(x_t[mi][:], x_t[mi][:], moe[mi][:])
            layer_norm(x_t, CL2G, CL2B, moe, 0, list(NCH))

            for (n0, nl) in NCH:
                for k in range(DK):
                    nc.sync.dma_start(
                        out=out_d[k * 128:(k + 1) * 128, n0:n0 + nl],
                        in_=moe[k][:, n0:n0 + nl])


    nc.compile()
    return nc


def _get_nc(**flags):
    key = tuple(sorted(flags.items()))
    if key not in _CACHE:
        _CACHE[key] = _build(**flags)
    return _CACHE[key]


def run(inputs, _flags=None, **spmd_kwargs):
    nc = _get_nc(**(_flags or {}))
    inp = {k: np.ascontiguousarray(np.asarray(v, dtype=np.float32))
           for k, v in inputs.items()}
    f32 = np.float32
    fp8 = ml_dtypes.float8_e4m3
    shared = {
        "Wq": inp["Wq"], "Wk": inp["Wk"], "Wv": inp["Wv"], "Wo": inp["Wo"],
        "bg": inp["bg"],
    }
    shared["colpack"] = np.ascontiguousarray(
        np.stack([inp["bq"], inp["bk"], inp["bo"], inp["bv"],
                  inp["ln1_g"], inp["ln1_b"], inp["ln2_g"], inp["ln2_b"]])
        .reshape(8, DK, 128).transpose(2, 0, 1))
    shared["wgp"] = np.ascontiguousarray(
        inp["Wg"].reshape(DK, 128, E).transpose(1, 0, 2))
    shared["b1T"] = np.ascontiguousarray(
        inp["b1"].reshape(E, FK, 128).transpose(0, 2, 1))
    shared["b2r"] = inp["b2"]
    shared["w1f8"] = np.ascontiguousarray(
        inp["W1"].reshape(E, DK, 128, F).transpose(0, 2, 1, 3).astype(fp8))
    shared["w2f8"] = np.ascontiguousarray(
        inp["W2"].reshape(E, FK, 128, D).transpose(0, 2, 1, 3).astype(fp8))
    sel = np.zeros((E, E * 128), dtype=f32)
    for e in range(E):
        sel[e, e * 128:(e + 1) * 128] = 1.0
    shared["sel8"] = sel
    ones = np.ones((128, 128), dtype=f32)
    shared["aux_ones"] = ones
    shared["aux_onesr"] = ones
    zo = np.zeros((2, 128), dtype=f32)
    zo[0, 0:64] = 1.0
    zo[1, 64:128] = 1.0
    shared["zo2"] = zo
    shared["aux_eps"] = np.full((1, 1), EPS, dtype=f32)
    in_maps = []
    for c in range(NCORES):
        m = dict(shared)
        for name in ("q", "k", "v"):
            m[name + "T"] = np.ascontiguousarray(
                inp[name][c * BPC:(c + 1) * BPC].reshape(T, D).T)
        in_maps.append(m)
    res = run_bass_kernel_spmd(nc, in_maps, core_ids=list(range(NCORES)),
                               **spmd_kwargs)
    out = np.stack([r["outT"].T for r in res.results])  # [8, T, D]
    return out.reshape(B, S, D), res


def kernel(**inputs):
    out, _ = run(inputs)
    return out


# revision 28
# speedup vs baseline: 1.0689x; 1.0689x over previous
"""MoE transformer block on 8 TRN2 NeuronCores.

Sharding: data-parallel over batch (4 batches = 784 tokens per core), no
collectives.  On-chip layout is feature-major ([d, tokens]) everywhere; the
host pre-transposes q/k/v to [D, T] and un-transposes the [D, T] output, so
the device does no layout transposes on the IO path.

MoE runs in fp8e4 DoubleRow (2 fp8 MACs/cell): the host pre-casts W1/W2 to
fp8 and pre-interleaves them into the [128, kpair, free] stationary layout,
so weight DMA is 37.7 MB/core of contiguous 18 KB/partition reads.  x and h
are cast to fp8 on-chip (DVE / ACT evictions).  Attention stays fp32r.

PSUM discipline: two pools, one unified tag each (every psum tile <= 1 bank,
4 bufs per pool -> exactly 8 banks).  The MoE y-phase holds 2+2 accumulators
across the K(=F) loop while the h-phase double-buffers 1+1.
"""
import sys

sys.path.insert(0, "/opt/trn_rl_repo")

from contextlib import ExitStack

import ml_dtypes
import numpy as np

import concourse.bass as bass
import concourse.tile as tile
from concourse import bacc, mybir
from concourse.bass_utils import run_bass_kernel_spmd

FP32 = mybir.dt.float32
FP32R = mybir.dt.float32r
FP8 = mybir.dt.float8e4
BF16 = mybir.dt.bfloat16
DR = mybir.MatmulPerfMode.DoubleRow
AF = mybir.ActivationFunctionType
OP = mybir.AluOpType

B, S, D, H, E, F = 32, 196, 768, 12, 8, 3072
DH = D // H                 # 64
NCORES = 8
BPC = B // NCORES           # 4 batches per core
T = BPC * S                 # 784 tokens per core
TH = T // 2                 # 392 tokens per half (2 batches)
DK = D // 128               # 6
FK = F // 128               # 24
EPS = 1e-5
BK = [(0, 128), (128, 68)]                              # ki chunks per batch
NCH = ((0, TH), (TH, TH))                               # token halves
MCH = ((0, 512), (512, T - 512))                        # moe chunks (16B align)
# packed bias/scale column indices in colpack
CBQ, CBK, CBO, CBV, CL1G, CL1B, CL2G, CL2B, CBT = range(9)

_CACHE = {}


def _build(skip_attn=False, skip_moe=False, repeat=1):
    nc = bacc.Bacc("TRN2", target_bir_lowering=False, debug=False,
                   num_devices=NCORES)

    qT_d = nc.dram_tensor("qT", [D, T], FP32R, kind="ExternalInput").ap()
    kT_d = nc.dram_tensor("kT", [D, T], FP32R, kind="ExternalInput").ap()
    vT_d = nc.dram_tensor("vT", [D, T], FP32R, kind="ExternalInput").ap()
    wq_d = nc.dram_tensor("Wq", [D, D], FP32R, kind="ExternalInput").ap()
    wk_d = nc.dram_tensor("Wk", [D, D], FP32R, kind="ExternalInput").ap()
    wv_d = nc.dram_tensor("Wv", [D, D], FP32R, kind="ExternalInput").ap()
    wo_d = nc.dram_tensor("Wo", [D, D], FP32R, kind="ExternalInput").ap()
    cols_d = nc.dram_tensor("colpack", [128, 9, DK], FP32,
                            kind="ExternalInput").ap()
    wgp_d = nc.dram_tensor("wgp", [128, DK, E], FP32R,
                           kind="ExternalInput").ap()
    bg_d = nc.dram_tensor("bg", [E], FP32, kind="ExternalInput").ap()
    b1t_d = nc.dram_tensor("b1T", [E, 128, FK], FP32,
                           kind="ExternalInput").ap()
    b2r_d = nc.dram_tensor("b2r", [E, D], FP32R, kind="ExternalInput").ap()
    w1f8_d = nc.dram_tensor("w1f8", [E, 128, DK, F], FP8,
                            kind="ExternalInput").ap()
    w2f8_d = nc.dram_tensor("w2f8", [E, 128, FK, D], FP8,
                            kind="ExternalInput").ap()
    sel_d = nc.dram_tensor("sel8", [E, E * 128], FP32R,
                           kind="ExternalInput").ap()
    aux1_d = nc.dram_tensor("aux_ones", [128, 128], FP32,
                            kind="ExternalInput").ap()
    aux1r_d = nc.dram_tensor("aux_onesr", [128, 128], FP32R,
                             kind="ExternalInput").ap()
    zo_d = nc.dram_tensor("zo2", [2, 128], FP32R, kind="ExternalInput").ap()
    auxe_d = nc.dram_tensor("aux_eps", [1, 1], FP32,
                            kind="ExternalInput").ap()
    out_d = nc.dram_tensor("outT", [D, T], FP32, kind="ExternalOutput").ap()

    with tile.TileContext(nc) as tc, ExitStack() as top:
        const = top.enter_context(tc.tile_pool(name="const", bufs=1))
        vecs = top.enter_context(tc.tile_pool(name="vecs", bufs=1))
        rows = top.enter_context(tc.tile_pool(name="rows", bufs=2))
        psA = top.enter_context(tc.tile_pool(name="psA", bufs=4, space="PSUM"))
        psB = top.enter_context(tc.tile_pool(name="psB", bufs=4, space="PSUM"))
        tmp = top.enter_context(tc.tile_pool(name="tmp", bufs=2))
        persist = top.enter_context(tc.tile_pool(name="persist", bufs=1))

        def pa(p, f):
            return psA.tile([p, f], FP32, tag="a", name="pa")

        def pb(p, f):
            return psB.tile([p, f], FP32, tag="b", name="pb")

        # ---------------- constants ----------------
        ones_col_r = const.tile([128, 1], FP32R, tag="ones_col_r")
        nc.sync.dma_start(out=ones_col_r[:], in_=aux1r_d[:, 0:1])
        ones_row_r = const.tile([1, 128], FP32R, tag="ones_row_r")
        nc.sync.dma_start(out=ones_row_r[:], in_=aux1r_d[0:1, :])
        ones_row8_r = const.tile([1, 8], FP32R, tag="ones_row8_r")
        nc.sync.dma_start(out=ones_row8_r[:], in_=aux1r_d[0:1, 0:8])
        ones8_col = const.tile([8, 1], FP32, tag="ones8_col")
        nc.sync.dma_start(out=ones8_col[:], in_=aux1_d[0:8, 0:1])
        # mask rows for the 1/den broadcast: row0 -> cols 0:64, row32 -> 64:128
        zo33 = const.tile([33, 128], FP32R, tag="zo33")
        nc.sync.dma_start(out=zo33[0:1, :], in_=zo_d[0:1, :])
        nc.sync.dma_start(out=zo33[32:33, :], in_=zo_d[1:2, :])
        eps_t = const.tile([1, 1], FP32, tag="eps")
        nc.sync.dma_start(out=eps_t[:], in_=auxe_d[:, :])
        # per-expert selector: sel8[i, e*128 + p] = (i == e), host-built
        sel8 = const.tile([8, E * 128], FP32R, tag="sel8")
        nc.sync.dma_start(out=sel8[:], in_=sel_d[:, :])

        # packed bias/scale columns: cp[p, i, k] = vec_i[k*128 + p]
        cp = vecs.tile([128, 9, DK], FP32, tag="colpack")
        nc.sync.dma_start(out=cp[:], in_=cols_d[:, :, :])

        def col(i, k):
            return cp[:, i, k:k + 1]

        bg_col = vecs.tile([8, 1], FP32, tag="bg")
        nc.sync.dma_start(out=bg_col[:],
                          in_=bg_d.rearrange("(a b) -> a b", b=1))
        wgs = vecs.tile([128, DK, E], FP32R, tag="wg")
        nc.sync.dma_start(out=wgs[:], in_=wgp_d[:, :, :])
        b2s = vecs.tile([E, D], FP32R, tag="b2")
        nc.sync.dma_start(out=b2s[:], in_=b2r_d[:, :])

        # persistent activations (full T)
        x_t = [persist.tile([128, T], FP32R, tag=f"xt{k}", name=f"xt{k}") for k in range(DK)]
        moe = [persist.tile([128, T], FP32, tag=f"moe{k}", name=f"moe{k}") for k in range(DK)]
        gexp = persist.tile([8, T], FP32, tag="gexp")
        gate = persist.tile([8, T], FP32R, tag="gate")

        def layer_norm(r_tiles, gi, bi, out_tiles, out_off, nch_list):
            # feature-major LN over D=768 partitions (6 tiles); r_tiles fp32r
            for (n0, nl) in nch_list:
                ps_s = pa(1, TH)
                ps_s2 = pa(1, TH)
                sqs = []
                for k in range(DK):
                    sq = tmp.tile([128, TH], FP32R, tag="ln_sq", bufs=6)
                    nc.scalar.activation(sq[:], r_tiles[k][:, n0:n0 + nl],
                                         AF.Square)
                    sqs.append(sq)
                for k in range(DK):
                    nc.tensor.matmul(ps_s[:], ones_col_r[:],
                                     r_tiles[k][:, n0:n0 + nl],
                                     start=(k == 0), stop=(k == DK - 1))
                for k in range(DK):
                    nc.tensor.matmul(ps_s2[:], ones_col_r[:], sqs[k][:],
                                     start=(k == 0), stop=(k == DK - 1))
                m = rows.tile([1, TH], FP32, tag="ln_m", bufs=1)
                m2 = rows.tile([1, TH], FP32, tag="ln_m2", bufs=1)
                nc.vector.tensor_scalar_mul(m[:], ps_s[:], 1.0 / D)
                nc.vector.tensor_scalar_mul(m2[:], ps_s2[:], 1.0 / D)
                mm_ = rows.tile([1, TH], FP32, tag="ln_mm", bufs=1)
                nc.vector.tensor_mul(mm_[:], m[:], m[:])
                var = rows.tile([1, TH], FP32, tag="ln_var", bufs=1)
                nc.vector.tensor_sub(var[:], m2[:], mm_[:])
                sd = rows.tile([1, TH], FP32, tag="ln_sd", bufs=1)
                nc.scalar.activation(sd[:], var[:], AF.Sqrt, bias=eps_t[:])
                rstd = rows.tile([1, TH], FP32R, tag="ln_rstd", bufs=1)
                with nc.allow_low_precision(reason="fp32r matmul operand"):
                    nc.vector.reciprocal(rstd[:], sd[:])
                mr = rows.tile([1, TH], FP32R, tag="ln_mr", bufs=1)
                nc.vector.tensor_mul(mr[:], m[:], rstd[:])
                pR = pb(128, TH)
                nc.tensor.matmul(pR[:], ones_row_r[:], rstd[:],
                                 start=True, stop=True)
                pM = pb(128, TH)
                nc.tensor.matmul(pM[:], ones_row_r[:], mr[:],
                                 start=True, stop=True)
                for k in range(DK):
                    t1 = tmp.tile([128, TH], FP32, tag="ln_t1")
                    nc.vector.tensor_mul(t1[:], r_tiles[k][:, n0:n0 + nl],
                                         pR[:])
                    t2 = tmp.tile([128, TH], FP32, tag="ln_t2")
                    nc.vector.tensor_sub(t2[:], t1[:], pM[:])
                    o0 = out_off + n0
                    nc.scalar.activation(out_tiles[k][:, o0:o0 + nl],
                                         t2[:], AF.Identity,
                                         bias=col(bi, k),
                                         scale=col(gi, k))

        def gates_chunk(n0, nl):
            pg = pb(8, TH)
            for k in range(DK):
                nc.tensor.matmul(pg[:], wgs[:, k, :], x_t[k][:, n0:n0 + nl],
                                 start=(k == 0), stop=(k == DK - 1))
            nc.scalar.activation(gexp[:, n0:n0 + nl], pg[:], AF.Exp,
                                 bias=bg_col[:])
            pgs = pb(1, TH)
            nc.tensor.matmul(pgs[:], ones8_col[:], gexp[:, n0:n0 + nl],
                             start=True, stop=True)
            grec = rows.tile([1, TH], FP32R, tag="grec", bufs=1)
            with nc.allow_low_precision(reason="fp32r matmul operand"):
                nc.vector.reciprocal(grec[:], pgs[:])
            pgr = pb(8, TH)
            nc.tensor.matmul(pgr[:], ones_row8_r[:], grec[:],
                             start=True, stop=True)
            nc.vector.tensor_mul(gate[:, n0:n0 + nl], gexp[:, n0:n0 + nl],
                                 pgr[:])
            # moe_acc init = gates^T @ b2   (lhsT = b2 chunks [8, 128])
            for mi in range(DK):
                pbi = pa(128, TH)
                nc.tensor.matmul(pbi[:], b2s[:, mi * 128:(mi + 1) * 128],
                                 gate[:, n0:n0 + nl], start=True, stop=True)
                nc.scalar.copy(moe[mi][:, n0:n0 + nl], pbi[:])

        for rep_i in range(repeat):
            # ================= attention, per token-half =================
            if skip_attn:
                for k in range(DK):
                    nc.sync.dma_start(out=x_t[k][:],
                                      in_=qT_d[k * 128:(k + 1) * 128, :])
                for (n0, nl) in NCH:
                    gates_chunk(n0, nl)
            for half in range(2 if not skip_attn else 0):
                h0tok = half * TH
                with ExitStack() as hs:
                    ph = hs.enter_context(tc.tile_pool(name=f"ph{half}_{rep_i}", bufs=1))
                    phw = hs.enter_context(tc.tile_pool(name=f"phw{half}_{rep_i}", bufs=6))
                    phe = hs.enter_context(tc.tile_pool(name=f"phe{half}_{rep_i}", bufs=4))

                    q_t = [ph.tile([128, TH], FP32R, tag=f"qt{k}", name=f"qt{k}")
                           for k in range(DK)]
                    k_t = [ph.tile([128, TH], FP32R, tag=f"kt{k}", name=f"kt{k}")
                           for k in range(DK)]
                    v_t = [ph.tile([128, TH], FP32R, tag=f"vt{k}", name=f"vt{k}")
                           for k in range(DK)]
                    # ---- loads, in dependency-priority order: the sync DMA
                    # ring is FIFO, so q+Wq go first (first matmuls need only
                    # them), then k+Wk, then v+Wv.
                    wq = []
                    wk = []
                    wv = []
                    for dram, dst, wdram, wlist in (
                            (qT_d, q_t, wq_d, wq), (kT_d, k_t, wk_d, wk),
                            (vT_d, v_t, wv_d, wv)):
                        for k in range(DK):
                            nc.sync.dma_start(
                                out=dst[k][:],
                                in_=dram[k * 128:(k + 1) * 128,
                                         h0tok:h0tok + TH])
                        for k in range(DK):
                            wt = phw.tile([128, D], FP32R, tag="wproj", name="wt")
                            nc.sync.dma_start(
                                out=wt[:], in_=wdram[k * 128:(k + 1) * 128, :])
                            wlist.append(wt)

                    # ---- qh/kh projections (fp32 out, feed fp32 scores) ----
                    qh_t = [ph.tile([128, TH], FP32R, tag=f"qh{k}", name=f"qh{k}")
                            for k in range(DK)]
                    kh_t = [ph.tile([128, TH], FP32R, tag=f"kh{k}", name=f"kh{k}")
                            for k in range(DK)]
                    for w, src, dst, bci in ((wq, q_t, qh_t, CBQ),
                                             (wk, k_t, kh_t, CBK)):
                        for mi in range(DK):
                            ps = pa(128, TH)
                            for k in range(DK):
                                nc.tensor.matmul(
                                    ps[:], w[k][:, mi * 128:(mi + 1) * 128],
                                    src[k][:], start=(k == 0), stop=(k == DK - 1))
                            nc.scalar.activation(dst[mi][:], ps[:], AF.Identity,
                                                 bias=col(bci, mi))

                    # ---- vh token-major per (batch, ki-chunk), zero-padded:
                    # head hh occupies cols (hh%2)*64..+64 of its 128-wide
                    # slice, the other half stays zero so even/odd heads can
                    # share a ctx PSUM bank via accumulation ----
                    vh = {}
                    for bl in range(2):
                        for ci, (c0, cl) in enumerate(BK):
                            vt_ = ph.tile([128, H, 128], FP32R,
                                          tag=f"vh{bl}{ci}", name=f"vh{bl}{ci}")
                            nc.vector.memset(
                                vt_[:].rearrange("p h f -> p (h f)")
                                .bitcast(FP32), 0.0)
                            tc0 = bl * S + c0
                            for ni in range(2):
                                ps = pa(128, 384)
                                for k in range(DK):
                                    nc.tensor.matmul(
                                        ps[:cl, :], v_t[k][:, tc0:tc0 + cl],
                                        wv[k][:, ni * 384:(ni + 1) * 384],
                                        start=(k == 0), stop=(k == DK - 1))
                                src = ps[:cl, :].rearrange(
                                    "p (j two d) -> p j two d", two=2, d=DH)
                                dst = vt_[:cl, ni * 6:(ni + 1) * 6, :] \
                                    .rearrange("p (j two) f -> p j two f",
                                               two=2)
                                for par in range(2):
                                    nc.vector.tensor_copy(
                                        dst[:, :, par, par * DH:(par + 1) * DH],
                                        src[:, :, par, :])
                            vh[(bl, ci)] = vt_

                    # ---- attention, head pairs sharing a ctx PSUM bank ----
                    # even head writes all 128 rows (values + zero half),
                    # odd head accumulates its half on top
                    cxp = [ph.tile([128, TH], FP32R, tag=f"cx{mi}", name=f"cx{mi}")
                           for mi in range(DK)]
                    for dm in range(DK):
                        ctxs = (pa(128, TH), pa(128, TH))
                        pdens = []
                        for sub in range(2):
                            hh = dm * 2 + sub
                            ro = sub * DH
                            pden = pb(1, TH)
                            for bl in range(2):
                                bc = bl * S
                                exps = []
                                for ci, (c0, cl) in enumerate(BK):
                                    # scores^T: batch bl keys vs BOTH batches'
                                    # queries (cross half garbage, never read)
                                    ps = pa(128, TH)
                                    nc.tensor.matmul(
                                        ps[:cl, :],
                                        kh_t[dm][ro:ro + DH,
                                                 bc + c0:bc + c0 + cl],
                                        qh_t[dm][ro:ro + DH, :],
                                        start=True, stop=True)
                                    ex = phe.tile([128, TH], FP32R, tag="exp",
                                                  bufs=3)
                                    nc.scalar.activation(ex[:cl, :], ps[:cl, :],
                                                         AF.Exp, scale=0.125)
                                    exps.append((ex, cl))
                                for ci, (ex, cl) in enumerate(exps):
                                    nc.tensor.matmul(
                                        ctxs[bl][:],
                                        vh[(bl, ci)][:cl, hh, :], ex[:cl, :],
                                        start=(sub == 0 and ci == 0),
                                        stop=(sub == 1 and ci == 1),
                                        skip_group_check=True)
                                for ci, (ex, cl) in enumerate(exps):
                                    nc.tensor.matmul(
                                        pden[0:1, bc:bc + S], ones_col_r[:cl],
                                        ex[:cl, bc:bc + S],
                                        start=(ci == 0), stop=(ci == 1),
                                        skip_group_check=True)
                            pdens.append(pden)
                        # evict unnormalized ctx so the PSUM banks free
                        # immediately; normalization trails on ACT/DVE
                        ctxu = []
                        for bl in range(2):
                            cu = phe.tile([128, S], FP32, tag="ctxu", bufs=3)
                            nc.scalar.copy(cu[:], ctxs[bl][:, bl * S:(bl + 1) * S])
                            ctxu.append(cu)
                        # both heads' denominators -> rows 0 / 32, broadcast
                        # via mask rows, one full-width reciprocal
                        dno = rows.tile([33, TH], FP32R, tag="dno", bufs=2)
                        nc.scalar.copy(dno[0:1, :], pdens[0][:])
                        nc.scalar.copy(dno[32:33, :], pdens[1][:])
                        pbc = pb(128, TH)
                        nc.tensor.matmul(pbc[:], zo33[0:1, :], dno[0:1, :],
                                         start=True, stop=False)
                        nc.tensor.matmul(pbc[:], zo33[32:33, :], dno[32:33, :],
                                         start=False, stop=True)
                        rdenb = phe.tile([128, TH], FP32, tag="rdenb", bufs=1)
                        nc.vector.reciprocal(rdenb[:], pbc[:])
                        for bl in range(2):
                            bc = bl * S
                            nc.vector.tensor_mul(cxp[dm][:, bc:bc + S],
                                                 ctxu[bl][:],
                                                 rdenb[:, bc:bc + S])

                    # ---- Wo projection + bias_total + residual -> r1 ----
                    wo = []
                    for k in range(DK):
                        wt = phw.tile([128, D], FP32R, tag="wproj", name="wt")
                        nc.sync.dma_start(
                            out=wt[:], in_=wo_d[k * 128:(k + 1) * 128, :])
                        wo.append(wt)
                    r1 = [ph.tile([128, TH], FP32R, tag=f"r1{mi}", name=f"r1{mi}")
                          for mi in range(DK)]
                    for mi in range(DK):
                        ps = pa(128, TH)
                        for k in range(DK):
                            nc.tensor.matmul(
                                ps[:], wo[k][:, mi * 128:(mi + 1) * 128],
                                cxp[k][:], start=(k == 0), stop=(k == DK - 1))
                        nc.vector.scalar_tensor_tensor(
                            out=r1[mi][:], in0=ps[:],
                            scalar=col(CBT, mi), in1=q_t[mi][:],
                            op0=OP.add, op1=OP.add)

                    layer_norm(r1, CL1G, CL1B, x_t, h0tok, [(0, TH)])
                    gates_chunk(h0tok, TH)

            # ================= MoE experts (fp8e4 DoubleRow) =================
            with ExitStack() as ms:
              if not skip_moe:
                  pmh = ms.enter_context(tc.tile_pool(name=f"pmh_{rep_i}", bufs=2))
                  pmw = ms.enter_context(tc.tile_pool(name=f"pmw_{rep_i}", bufs=2))
                  # x8[p, j, t] = x[j*128+p, t] pair-interleaved for DR rhs
                  x8 = pmh.tile([128, DK, T], FP8, tag="x8", bufs=1, name="x8")
                  with nc.allow_low_precision(reason="fp8 moe operand"):
                      for k in range(DK):
                          nc.vector.tensor_copy(x8[:, k, :],
                                                x_t[k][:].bitcast(FP32))
                  for e in range(E):
                      # weights pre-cast to fp8 + pre-interleaved on the host
                      w1t = pmw.tile([128, DK, F], FP8, tag="w1", bufs=2,
                                     name="w1t")
                      nc.scalar.dma_start(out=w1t[:], in_=w1f8_d[e])
                      w2t = pmw.tile([128, FK, D], FP8, tag="w2", bufs=2,
                                     name="w2t")
                      nc.scalar.dma_start(out=w2t[:], in_=w2f8_d[e])
                      b1c = rows.tile([128, FK], FP32, tag="b1col")
                      nc.sync.dma_start(out=b1c[:], in_=b1t_d[e])

                      # gate row broadcast to 128 partitions, evicted to SBUF
                      grep = tmp.tile([128, T], FP32, tag="gerep")
                      for (n0, nl) in NCH:
                          pge = pb(128, TH)
                          nc.tensor.matmul(pge[:],
                                           sel8[:, e * 128:(e + 1) * 128],
                                           gate[:, n0:n0 + nl],
                                           start=True, stop=True)
                          nc.vector.tensor_copy(grep[:, n0:n0 + nl], pge[:])

                      # ---- h = gelu(W1[e]^T @ x + b1), fp8 [128, FK, T] ----
                      h8 = pmh.tile([128, FK, T], FP8, tag="h8", bufs=1,
                                    name="h8")
                      for fm in range(FK):
                          ph0 = pa(128, MCH[0][1])
                          ph1 = pb(128, MCH[1][1])
                          phs = (ph0, ph1)
                          for j in range(DK // 2):
                              for ni, (n0, nl) in enumerate(MCH):
                                  nc.tensor.matmul(
                                      phs[ni][:],
                                      w1t[:, 2 * j:2 * j + 2,
                                          fm * 128:(fm + 1) * 128],
                                      x8[:, 2 * j:2 * j + 2, n0:n0 + nl],
                                      start=(j == 0), stop=(j == DK // 2 - 1),
                                      perf_mode=DR)
                          for ni, (n0, nl) in enumerate(MCH):
                              nc.scalar.activation(h8[:, fm, n0:n0 + nl],
                                                   phs[ni][:], AF.Gelu,
                                                   bias=b1c[:, fm:fm + 1])

                      # ---- y = W2[e]^T @ h (K-accum in PSUM), combine ----
                      for dg in range(3):
                          pys = [pa(128, MCH[0][1]), pa(128, MCH[1][1]),
                                 pb(128, MCH[0][1]), pb(128, MCH[1][1])]
                          for f2 in range(FK // 2):
                              for j in range(2):
                                  mi = dg * 2 + j
                                  for ni, (n0, nl) in enumerate(MCH):
                                      nc.tensor.matmul(
                                          pys[j * 2 + ni][:],
                                          w2t[:, 2 * f2:2 * f2 + 2,
                                              mi * 128:(mi + 1) * 128],
                                          h8[:, 2 * f2:2 * f2 + 2, n0:n0 + nl],
                                          start=(f2 == 0),
                                          stop=(f2 == FK // 2 - 1),
                                          perf_mode=DR)
                          for j in range(2):
                              mi = dg * 2 + j
                              for ni, (n0, nl) in enumerate(MCH):
                                  ty = tmp.tile([128, nl], FP32, tag="ty")
                                  nc.vector.tensor_mul(ty[:], pys[j * 2 + ni][:],
                                                       grep[:, n0:n0 + nl])
                                  nc.vector.tensor_add(moe[mi][:, n0:n0 + nl],
                                                       moe[mi][:, n0:n0 + nl],
                                                       ty[:])

            # ================= LN2 + output =================
            # r2 = x + moe, written in place into x_t; LN2 output reuses moe
            for (n0, nl) in NCH:
                for mi in range(DK):
                    nc.vector.tensor_add(x_t[mi][:, n0:n0 + nl],
                                         x_t[mi][:, n0:n0 + nl],
                                         moe[mi][:, n0:n0 + nl])
                layer_norm(x_t, CL2G, CL2B, moe, 0, [(n0, nl)])

            for (n0, nl) in NCH:
                for k in range(DK):
                    nc.sync.dma_start(
                        out=out_d[k * 128:(k + 1) * 128, n0:n0 + nl],
                        in_=moe[k][:, n0:n0 + nl])


    nc.compile()
    return nc


def _get_nc(**flags):
    key = tuple(sorted(flags.items()))
    if key not in _CACHE:
        _CACHE[key] = _build(**flags)
    return _CACHE[key]


def run(inputs, _flags=None, **spmd_kwargs):
    nc = _get_nc(**(_flags or {}))
    inp = {k: np.ascontiguousarray(np.asarray(v, dtype=np.float32))
           for k, v in inputs.items()}
    f32 = np.float32
    fp8 = ml_dtypes.float8_e4m3
    bf16 = ml_dtypes.bfloat16
    shared = {
        "Wq": inp["Wq"], "Wk": inp["Wk"], "Wv": inp["Wv"], "Wo": inp["Wo"],
        "bg": inp["bg"],
    }
    btot = inp["bo"] + inp["bv"] @ inp["Wo"]
    shared["colpack"] = np.ascontiguousarray(
        np.stack([inp["bq"], inp["bk"], inp["bo"], inp["bv"],
                  inp["ln1_g"], inp["ln1_b"], inp["ln2_g"], inp["ln2_b"],
                  btot]).reshape(9, DK, 128).transpose(2, 0, 1))
    shared["wgp"] = np.ascontiguousarray(
        inp["Wg"].reshape(DK, 128, E).transpose(1, 0, 2))
    shared["b1T"] = np.ascontiguousarray(
        inp["b1"].reshape(E, FK, 128).transpose(0, 2, 1))
    shared["b2r"] = inp["b2"]
    shared["w1f8"] = np.ascontiguousarray(
        inp["W1"].reshape(E, DK, 128, F).transpose(0, 2, 1, 3).astype(fp8))
    shared["w2f8"] = np.ascontiguousarray(
        inp["W2"].reshape(E, FK, 128, D).transpose(0, 2, 1, 3).astype(fp8))
    sel = np.zeros((E, E * 128), dtype=f32)
    for e in range(E):
        sel[e, e * 128:(e + 1) * 128] = 1.0
    shared["sel8"] = sel
    ones = np.ones((128, 128), dtype=f32)
    shared["aux_ones"] = ones
    shared["aux_onesr"] = ones
    zo = np.zeros((2, 128), dtype=f32)
    zo[0, 0:64] = 1.0
    zo[1, 64:128] = 1.0
    shared["zo2"] = zo
    shared["aux_eps"] = np.full((1, 1), EPS, dtype=f32)
    in_maps = []
    for c in range(NCORES):
        m = dict(shared)
        for name in ("q", "k", "v"):
            m[name + "T"] = np.ascontiguousarray(
                inp[name][c * BPC:(c + 1) * BPC].reshape(T, D).T)
        in_maps.append(m)
    res = run_bass_kernel_spmd(nc, in_maps, core_ids=list(range(NCORES)),
                               **spmd_kwargs)
    out = np.stack([r["outT"].T for r in res.results])  # [8, T, D]
    return out.reshape(B, S, D), res


def kernel(**inputs):
    out, _ = run(inputs)
    return out
